# revision 6
# baseline (speedup 1.0000x reference)
"""Barlow Twins loss on 8 trn2 NeuronCores.

Math: with A = normalize(z_a), B = normalize(z_b) (per-column, ddof=1) and
c = A.T @ B / N, the loss is

    loss = lam * sum(c**2) + sum_d [ (c_dd - 1)**2 - lam * c_dd**2 ]

The expensive term sum(c**2) = ||A.T B||_F^2 / N^2 = tr((A A.T)(B B.T)) / N^2
only needs the [N, N] Gram matrices Ga = A A.T, Gb = B B.T (N=256), which are
separable over column shards: Ga = sum_cores A_i A_i.T.  The diagonal c_dd is
an elementwise product + reduce.  So each core, holding a 1024-column slice in
[D_local, N] (transposed) layout, computes per-column mean/std along the free
axis, normalizes, accumulates partial Ga/Gb via PE matmuls (fp32r), and the
diagonal dot products via the vector engine.  The host sums the 8 partial
Grams and finishes the tiny [256,256] reduction in float64.
"""

import os
from contextlib import ExitStack

import numpy as np

N = 256
D = 8192
NCORES = 8
D_LOCAL = D // NCORES  # 1024
P = 128
NT = D_LOCAL // P  # 8 partition-tiles per tensor per core
LAMBDA = 0.005

_CACHE: dict = {}


def _build_program():
    import concourse.bacc as bacc
    import concourse.tile as tile
    from concourse import mybir

    f32 = mybir.dt.float32
    f32r = mybir.dt.float32r
    Alu = mybir.AluOpType
    Act = mybir.ActivationFunctionType
    X = mybir.AxisListType.X

    nc = bacc.Bacc("TRN2", target_bir_lowering=False, debug=False)

    za_t = nc.dram_tensor("za_t", [D_LOCAL, N], f32, kind="ExternalInput").ap()
    zb_t = nc.dram_tensor("zb_t", [D_LOCAL, N], f32, kind="ExternalInput").ap()
    ga = nc.dram_tensor("ga", [2, P, N], f32, kind="ExternalOutput").ap()
    gb = nc.dram_tensor("gb", [2, P, N], f32, kind="ExternalOutput").ap()
    qd = nc.dram_tensor("qd", [P, NT], f32, kind="ExternalOutput").ap()

    with tile.TileContext(nc) as tc, ExitStack() as ctx:
        data_pool = ctx.enter_context(tc.tile_pool(name="data", bufs=NT))
        sq_pool = ctx.enter_context(tc.tile_pool(name="sq", bufs=3))
        st_pool = ctx.enter_context(tc.tile_pool(name="st", bufs=4))
        out_pool = ctx.enter_context(tc.tile_pool(name="out", bufs=1))
        ps_pool = ctx.enter_context(tc.tile_pool(name="ps", bufs=1, space="PSUM"))

        ga_ps = [ps_pool.tile([P, N], f32, tag=f"ga{m}", name=f"ga_ps{m}") for m in range(2)]
        gb_ps = [ps_pool.tile([P, N], f32, tag=f"gb{m}", name=f"gb_ps{m}") for m in range(2)]
        q_sb = out_pool.tile([P, NT], f32, tag="q", name="q_sb")
        ga_sb = out_pool.tile([P, 2, N], f32, tag="ga_sb", name="ga_sb")
        gb_sb = out_pool.tile([P, 2, N], f32, tag="gb_sb", name="gb_sb")

        for j in range(NT):
            tiles = {}
            for name, src in (("a", za_t), ("b", zb_t)):
                t = data_pool.tile([P, N], f32, tag=f"z{name}", name=f"z{name}{j}")
                nc.sync.dma_start(t[:], src[j * P : (j + 1) * P, :])

                s = st_pool.tile([P, 1], f32, tag=f"s{name}", name=f"s{name}{j}")
                nc.vector.reduce_sum(s[:], t[:], axis=X)

                sq = sq_pool.tile([P, N], f32, tag="sq", name=f"sq{name}{j}")
                ssq = st_pool.tile([P, 1], f32, tag=f"ssq{name}", name=f"ssq{name}{j}")
                nc.scalar.activation(sq[:], t[:], Act.Square, accum_out=ssq[:])

                mean = st_pool.tile([P, 1], f32, tag=f"mean{name}", name=f"mean{name}{j}")
                nc.vector.tensor_scalar_mul(mean[:], s[:], 1.0 / N)
                # t0 = s * mean = s^2 / N
                t0 = st_pool.tile([P, 1], f32, tag=f"t0{name}", name=f"t0{name}{j}")
                nc.vector.tensor_tensor(t0[:], s[:], mean[:], op=Alu.mult)
                # var = (ssq - s^2/N) / (N-1)   (unbiased, matches std(ddof=1))
                var = st_pool.tile([P, 1], f32, tag=f"var{name}", name=f"var{name}{j}")
                nc.vector.tensor_scalar(
                    out=var[:],
                    in0=ssq[:],
                    scalar1=t0[:],
                    scalar2=1.0 / (N - 1),
                    op0=Alu.subtract,
                    op1=Alu.mult,
                )
                iv = st_pool.tile([P, 1], f32, tag=f"iv{name}", name=f"iv{name}{j}")
                nc.vector.reciprocal(iv[:], var[:])
                istd = st_pool.tile([P, 1], f32, tag=f"istd{name}", name=f"istd{name}{j}")
                nc.scalar.sqrt(istd[:], iv[:])
                # normalize into an fp32r tile: (t - mean) * istd
                # (producer must round to fp32r for the PE matmuls)
                nt = data_pool.tile([P, N], f32r, tag=f"n{name}", name=f"n{name}{j}")
                nc.vector.tensor_scalar(
                    out=nt[:],
                    in0=t[:],
                    scalar1=mean[:],
                    scalar2=istd[:],
                    op0=Alu.subtract,
                    op1=Alu.mult,
                )
                tiles[name] = nt

            first, last = j == 0, j == NT - 1
            for m in range(2):
                nc.tensor.matmul(
                    ga_ps[m][:],
                    lhsT=tiles["a"][:, m * P : (m + 1) * P],
                    rhs=tiles["a"][:],
                    start=first,
                    stop=last,
                )
                nc.tensor.matmul(
                    gb_ps[m][:],
                    lhsT=tiles["b"][:, m * P : (m + 1) * P],
                    rhs=tiles["b"][:],
                    start=first,
                    stop=last,
                )

            # diagonal: q[:, j] = sum_n a*b
            # (tensor_tensor_reduce crashes TRN2 here; use mult + reduce)
            dsc = sq_pool.tile([P, N], f32, tag="dsc", name=f"dsc{j}")
            nc.vector.tensor_tensor(
                dsc[:],
                tiles["a"][:].bitcast(f32),
                tiles["b"][:].bitcast(f32),
                op=Alu.mult,
            )
            nc.vector.reduce_sum(q_sb[:, j : j + 1], dsc[:], axis=X)

        for m in range(2):
            nc.vector.tensor_copy(ga_sb[:, m, :], ga_ps[m][:])
            nc.vector.tensor_copy(gb_sb[:, m, :], gb_ps[m][:])
        for m in range(2):
            nc.sync.dma_start(ga[m], ga_sb[:, m, :])
            nc.sync.dma_start(gb[m], gb_sb[:, m, :])
        nc.sync.dma_start(qd[:], q_sb[:])

    nc.compile()
    return nc


def _get_program():
    if "nc" not in _CACHE:
        _CACHE["nc"] = _build_program()
    return _CACHE["nc"]


LAST_RESULT = None


def kernel(z_a: np.ndarray, z_b: np.ndarray) -> np.ndarray:
    global LAST_RESULT
    from concourse.bass_utils import run_bass_kernel_spmd

    z_a = np.asarray(z_a, dtype=np.float32)
    z_b = np.asarray(z_b, dtype=np.float32)
    assert z_a.shape == (N, D) and z_b.shape == (N, D)

    nc = _get_program()

    in_maps = []
    for c in range(NCORES):
        sl = slice(c * D_LOCAL, (c + 1) * D_LOCAL)
        in_maps.append(
            {
                "za_t": np.ascontiguousarray(z_a[:, sl].T),
                "zb_t": np.ascontiguousarray(z_b[:, sl].T),
            }
        )

    res = run_bass_kernel_spmd(nc, in_maps, core_ids=list(range(NCORES)))
    LAST_RESULT = res

    Ga = np.zeros((2 * P, N), dtype=np.float64)
    Gb = np.zeros((2 * P, N), dtype=np.float64)
    q = np.empty(D, dtype=np.float64)
    for c in range(NCORES):
        out = res.results[c]
        Ga += out["ga"].reshape(2 * P, N).astype(np.float64)
        Gb += out["gb"].reshape(2 * P, N).astype(np.float64)
        # qd[p, j] holds the diagonal dot for local column j*P + p
        q[c * D_LOCAL : (c + 1) * D_LOCAL] = (
            out["qd"].T.reshape(D_LOCAL).astype(np.float64)
        )

    sum_c2 = float((Ga * Gb).sum()) / (N * N)  # sum over ALL (d, e) of c^2
    cdd = q / N
    loss = (
        LAMBDA * (sum_c2 - float((cdd * cdd).sum()))
        + float(((cdd - 1.0) ** 2).sum())
    )
    return np.float32(loss)


if __name__ == "__main__":
    rng = np.random.default_rng(0)
    za = rng.standard_normal((N, D), dtype=np.float32)
    zb = rng.standard_normal((N, D), dtype=np.float32)
    out = kernel(z_a=za, z_b=zb)
    print("kernel output:", out)
